# revision 16
# baseline (speedup 1.0000x reference)
"""Multi-head self-attention with LoRA projections on 8 Trainium2 NeuronCores.

Problem: nn_MultiHeadSelfAttention (B=2, L=2048, D=1024, H=16, hd=64, LoRA r=16).

Sharding (ZERO-collective): query-token parallel. Core c owns the 512 query
tokens [qb*512,(qb+1)*512) of batch b, where b = c//4, qb = c%4. Each core
computes K/V for its whole batch locally (replicated across the 4 cores of
that batch group) — this trades ~55us of extra PE time for eliminating the
AllToAll collective (multiple ms in this environment) and makes the final
output a clean per-core concat. Weights/LoRA factors are replicated.

Per-core pipeline (bf16 on the PE, fp32 accumulation in PSUM):
  1. Weff_p = W_p^T + 0.5*A_p@B_p folded on-chip: W^T arrives as a host
     layout-transpose, the rank-16 LoRA product is a PE matmul accumulated
     in PSUM and added in-place on DVE. Attention scale 1/8 at Q eviction.
  2. kT = Weff_k^T x^T in [out, tok] layout (+bk at ACT eviction), streamed
     by 512-token x chunks; qT likewise for the core's own 512 tokens; V in
     [tok, out] layout with a ones column per head (softmax row sums); bv
     deferred past softmax (softmax rows sum to 1).
  3. Attention runs in HEAD PAIRS with two interleaved dependency chains:
     per key tile, S^T [m=128, l=512] in PSUM (contract hd=64); bias
     (host-pre-transposed [m,l] bf16) added on DVE for most tiles and
     PE-injected (identity-matmul accumulate) for 5/16 tiles to balance
     engines; one 1024-wide exp per pair on ACT; AV accumulates O'^T
     [65, 512] in half-chains (row 64 = softmax denominator) so the PSUM
     ring stays small. V's second half and the o-projection weight prep run
     as PE filler between pairs.
  4. Finalize per head: PE-transpose + DVE reciprocal normalize, transpose
     back, +bv; O^T overwrites the dead qT region (no extra SBUF).
  5. y = O^T.T @ Weff_o + bo (bo via rank-1 ones matmul), fp32 out.

Host side only shards/casts/layout-transposes/concats: slices x/bias per
core, casts to bf16, pre-transposes W/A/x/bias (layout only — all arithmetic
including the LoRA fold stays on device), concatenates the per-core
[512, 1024] fp32 outputs into [2, 2048, 1024].
"""

import numpy as np
import ml_dtypes

BF16 = ml_dtypes.bfloat16

B = 2
L = 2048
D = 1024
H = 16
HD = 64
R = 16
SCALING = 0.5  # LoRA alpha/r
SCALE = HD ** (-0.5)  # attention scale, applied at Q eviction

N_CORES = 8
QPC = 512  # query tokens per core
KT = D // 128  # 8 contraction tiles
MT = L // 128  # 16 key tiles per batch
LTQ = QPC // 128  # 4 query tiles per core

_CACHE = {}


def _build_kernel(num_devices=N_CORES, repeat=1):
    import concourse.tile as tile
    import concourse.mybir as mybir
    from concourse import bacc
    from concourse.masks import make_identity
    from contextlib import ExitStack

    f32 = mybir.dt.float32
    bf16 = mybir.dt.bfloat16
    AF = mybir.ActivationFunctionType
    ALU = mybir.AluOpType

    nc = bacc.Bacc("TRN2", target_bir_lowering=False, debug=False,
                   enable_asserts=False, num_devices=num_devices)

    # ---- per-core external inputs (bf16 pre-cast / layout-prepped on host) --
    xbT_ap = nc.dram_tensor("xbT", [D, L], bf16, kind="ExternalInput").ap()
    xqT_ap = nc.dram_tensor("xqT", [D, QPC], bf16, kind="ExternalInput").ap()
    biasT_ap = nc.dram_tensor("biasT", [H, L, QPC], bf16,
                              kind="ExternalInput").ap()
    wt_aps, at_aps, lb_aps = {}, {}, {}
    for p in "qkvo":
        wt_aps[p] = nc.dram_tensor(f"WT{p}", [D, D], bf16,
                                   kind="ExternalInput").ap()
        at_aps[p] = nc.dram_tensor(f"AT{p}", [R, D], bf16,
                                   kind="ExternalInput").ap()
        lb_aps[p] = nc.dram_tensor(f"B{p}", [R, D], bf16,
                                   kind="ExternalInput").ap()
    bq_ap = nc.dram_tensor("bq", [D, 1], f32, kind="ExternalInput").ap()
    bk_ap = nc.dram_tensor("bk", [D, 1], f32, kind="ExternalInput").ap()
    bv2_ap = nc.dram_tensor("bv2", [D, 1], f32, kind="ExternalInput").ap()
    bo_ap = nc.dram_tensor("bo", [1, D], f32, kind="ExternalInput").ap()

    y_ap = nc.dram_tensor("y", [QPC, D], bf16, kind="ExternalOutput").ap()

    with tile.TileContext(nc) as tc, ExitStack() as top:
        const_pool = top.enter_context(tc.tile_pool(name="const", bufs=1))
        ident = const_pool.tile([128, 128], bf16)
        make_identity(nc, ident[:])
        identf = const_pool.tile([128, 128], f32)
        make_identity(nc, identf[:])
        ones_row = const_pool.tile([1, 128], bf16)
        nc.gpsimd.memset(ones_row[:], 1.0)
        bias_vec = const_pool.tile([128, KT, 3], f32)
        # bias_vec[:, kt, 0..2] = bq*SCALE | bk | bv
        bo_row = const_pool.tile([1, D], bf16)

        for rep in range(repeat):
          with ExitStack() as rctx:
            qkv_pool = rctx.enter_context(tc.tile_pool(name="qkv", bufs=1))
            kT = qkv_pool.tile([128, KT, L], bf16)          # K^T [out, tok]
            vsb = qkv_pool.tile([128, MT, H * 65], bf16)    # V [tok, h|1]
            qT = qkv_pool.tile([128, KT, QPC], bf16)  # Q^T; becomes O^T+bv

            weff_pool = rctx.enter_context(tc.tile_pool(name="weff", bufs=3))
            lsm = rctx.enter_context(tc.tile_pool(name="lsm", bufs=1))
            lora_sm = rctx.enter_context(tc.tile_pool(name="lora", bufs=2))
            xts = rctx.enter_context(tc.tile_pool(name="xts", bufs=2))
            bias_pool = rctx.enter_context(tc.tile_pool(name="bias", bufs=5))
            sadd_pool = rctx.enter_context(tc.tile_pool(name="sadd", bufs=3))
            e_pool = rctx.enter_context(tc.tile_pool(name="e", bufs=4))
            fin_pool = rctx.enter_context(tc.tile_pool(name="fin", bufs=4))
            fin2_pool = rctx.enter_context(tc.tile_pool(name="fin2", bufs=2))
            y_pool = rctx.enter_context(tc.tile_pool(name="ysb", bufs=1))
            sc = rctx.enter_context(tc.tile_pool(name="scps", bufs=4,
                                                 space="PSUM"))
            mm = rctx.enter_context(tc.tile_pool(name="mmps", bufs=2,
                                                 space="PSUM"))
            po_pool = rctx.enter_context(tc.tile_pool(name="pops", bufs=2,
                                                      space="PSUM"))

            # vsb ones columns (disjoint from V evictions; runs immediately)
            ones_cols = vsb[:].rearrange("p m (h e) -> p m h e", e=65)
            nc.gpsimd.memset(ones_cols[:, :, :, 64:65], 1.0)

            ats, lb, weff = {}, {}, {}

            def lora_factors(p, eng=None):
                eng = eng or nc.sync
                lb[p] = lora_sm.tile([R, D], bf16, tag="lb", name=f"lb{p}{rep}")
                eng.dma_start(lb[p][:], lb_aps[p][:, :])
                araw = lora_sm.tile([R, D], bf16, tag="araw", name=f"ar{p}{rep}")
                eng.dma_start(araw[:], at_aps[p][:, :])
                ats[p] = lora_sm.tile([R, D], bf16, tag="ats", name=f"at{p}{rep}")
                nc.gpsimd.tensor_scalar_mul(ats[p][:], araw[:], SCALING)

            for p in "kqv":
                lora_factors(p)

            # ---- small DMAs ----
            braw = lsm.tile([128, KT, 3], f32, name=f"braw{rep}")
            nc.sync.dma_start(
                braw[:, :, 0:1], bq_ap.rearrange("(kt p) o -> p kt o", p=128))
            nc.sync.dma_start(
                braw[:, :, 1:2], bk_ap.rearrange("(kt p) o -> p kt o", p=128))
            nc.sync.dma_start(
                braw[:, :, 2:3], bv2_ap.rearrange("(kt p) o -> p kt o", p=128))
            nc.gpsimd.dma_start(bo_row[:], bo_ap[:, :])  # cast f32->bf16
            nc.vector.tensor_scalar_mul(bias_vec[:, :, 0:1],
                                        braw[:, :, 0:1], SCALE)
            nc.vector.tensor_copy(bias_vec[:, :, 1:3], braw[:, :, 1:3])

            def weff_dma(p):
                weff[p] = weff_pool.tile([128, KT, D], bf16, tag="we",
                                         name=f"we{p}{rep}")
                for ki in range(KT):
                    ksl = slice(ki * 128, (ki + 1) * 128)
                    nc.sync.dma_start(
                        weff[p][:, ki, :],
                        wt_aps[p][ksl, :].rearrange("(o p2) c -> p2 o c",
                                                    p2=128))

            def weff_fold(p):
                for ki in range(KT):
                    ksl = slice(ki * 128, (ki + 1) * 128)
                    for oc in range(2):
                        osl = slice(oc * 512, (oc + 1) * 512)
                        ps = mm.tile([128, 512], f32, tag="mm",
                                     name=f"wf{p}{ki}{oc}")
                        nc.tensor.matmul(ps[:], ats[p][:, ksl], lb[p][:, osl])
                        nc.vector.scalar_tensor_tensor(
                            weff[p][:, ki, osl], ps[:], 1.0,
                            weff[p][:, ki, osl], ALU.mult, ALU.add)

            # DMA queue order tuned so PE never waits long: Wk, x-chunk0,
            # xq, Wq, Wv, then remaining x chunks
            weff_dma("k")
            xc0 = xts.tile([128, KT, 512], bf16, tag="xc", name=f"xc0{rep}")
            nc.sync.dma_start(
                xc0[:], xbT_ap[:, 0:512].rearrange("(ki p2) t -> p2 ki t",
                                                   p2=128))
            xqT = xts.tile([128, KT, QPC], bf16, tag="xc", name=f"xq{rep}")
            nc.sync.dma_start(
                xqT[:], xqT_ap.rearrange("(ki p2) t -> p2 ki t", p2=128))
            weff_dma("q")
            weff_dma("v")

            weff_fold("k")

            def k_chunk(tcc, xTc):
                for ot in range(KT):
                    osl = slice(ot * 128, (ot + 1) * 128)
                    ps = mm.tile([128, 512], f32, tag="mm",
                                 name=f"pk{tcc}{ot}")
                    for ki in range(KT):
                        nc.tensor.matmul(ps[:], weff["k"][:, ki, osl],
                                         xTc[:, ki, :],
                                         start=(ki == 0), stop=(ki == KT - 1),
                                         skip_group_check=True)
                    nc.scalar.add(kT[:, ot, tcc * 512:(tcc + 1) * 512],
                                  ps[:], bias_vec[:, ot, 1:2])

            def v_chunk(tcc, xTc, oc):
                for tw in range(4):
                    tt = tcc * 4 + tw
                    twsl = slice(tw * 128, (tw + 1) * 128)
                    ps = mm.tile([128, 512], f32, tag="mm",
                                 name=f"pv{tt}{oc}")
                    for ki in range(KT):
                        nc.tensor.matmul(
                            ps[:], xTc[:, ki, twsl],
                            weff["v"][:, ki, oc * 512:(oc + 1) * 512],
                            start=(ki == 0), stop=(ki == KT - 1),
                            skip_group_check=True)
                    dst = vsb[:, tt, oc * 520:(oc + 1) * 520].rearrange(
                        "p (h e) -> p h e", e=65)
                    nc.scalar.copy(
                        dst[:, :, 0:64],
                        ps[:].rearrange("p (h e) -> p h e", e=64))

            k_chunk(0, xc0)
            weff_fold("q")
            for ot in range(KT):  # Q^T (scale + bq at ACT eviction)
                osl = slice(ot * 128, (ot + 1) * 128)
                ps = mm.tile([128, QPC], f32, tag="mm", name=f"pq{ot}")
                for ki in range(KT):
                    nc.tensor.matmul(ps[:], weff["q"][:, ki, osl],
                                     xqT[:, ki, :],
                                     start=(ki == 0), stop=(ki == KT - 1),
                                     skip_group_check=True)
                nc.scalar.activation(qT[:, ot, :], ps[:], AF.Identity,
                                     bias=bias_vec[:, ot, 0:1], scale=SCALE)
            weff_fold("v")
            v_chunk(0, xc0, 0)
            for tcc in range(1, 4):
                xTc = xts.tile([128, KT, 512], bf16, tag="xc",
                               name=f"xcA{tcc}")
                nc.sync.dma_start(
                    xTc[:],
                    xbT_ap[:, tcc * 512:(tcc + 1) * 512]
                    .rearrange("(ki p2) t -> p2 ki t", p2=128))
                k_chunk(tcc, xTc)
                v_chunk(tcc, xTc, 0)

            # ---- deferred filler jobs (run interleaved between heads) ----
            vstate = {}

            def v_oc1(tt):
                def run():
                    tcc, tw = divmod(tt, 4)
                    if tw == 0:
                        xc = xts.tile([128, KT, 512], bf16, tag="xc",
                                      name=f"xc1{tt}")
                        nc.gpsimd.dma_start(
                            xc[:],
                            xbT_ap[:, tcc * 512:(tcc + 1) * 512]
                            .rearrange("(ki p2) t -> p2 ki t", p2=128))
                        vstate["xc"] = xc
                    xc = vstate["xc"]
                    twsl = slice(tw * 128, (tw + 1) * 128)
                    ps = mm.tile([128, 512], f32, tag="mm", name=f"pw{tt}")
                    for ki in range(KT):
                        nc.tensor.matmul(ps[:], xc[:, ki, twsl],
                                         weff["v"][:, ki, 512:1024],
                                         start=(ki == 0), stop=(ki == KT - 1),
                                         skip_group_check=True)
                    dst = vsb[:, tt, 520:1040].rearrange("p (h e) -> p h e",
                                                         e=65)
                    nc.vector.tensor_copy(
                        dst[:, :, 0:64],
                        ps[:].rearrange("p (h e) -> p h e", e=64))
                return run

            def wo_load_fold_piece(ki):
                def run():
                    if ki == 0:
                        lora_factors("o", nc.gpsimd)
                        weff["o"] = weff_pool.tile([128, KT, D], bf16,
                                                   tag="we", name=f"weo{rep}")
                    ksl = slice(ki * 128, (ki + 1) * 128)
                    nc.gpsimd.dma_start(
                        weff["o"][:, ki, :],
                        wt_aps["o"][ksl, :].rearrange("(o p2) c -> p2 o c",
                                                      p2=128))
                    for oc in range(2):
                        osl = slice(oc * 512, (oc + 1) * 512)
                        ps = mm.tile([128, 512], f32, tag="mm",
                                     name=f"wo{ki}{oc}")
                        nc.tensor.matmul(ps[:], ats["o"][:, ksl],
                                         lb["o"][:, osl])
                        nc.vector.scalar_tensor_tensor(
                            weff["o"][:, ki, osl], ps[:], 1.0,
                            weff["o"][:, ki, osl], ALU.mult, ALU.add)
                return run

            fillers = [v_oc1(tt) for tt in range(MT)]
            fillers += [wo_load_fold_piece(ki) for ki in range(KT)]

            # ---- attention: head PAIRS interleaved (two chains in flight),
            # AV split into half-accumulators so PSUM po ring stays at 2 ----
            def bias_dma(h, q):
                bt = bias_pool.tile([128, 4, QPC], bf16, tag="bn",
                                    name=f"bn{h}{q}")
                nc.sync.dma_start(
                    bt[:],
                    biasT_ap[h][q * 512:(q + 1) * 512, :]
                    .rearrange("(mt p) l -> p mt l", p=128))
                return bt

            def finalize_pieces(hp, hstages):
                # small closures (ssum per head, then per-j normalize
                # chains), emitted spread across the NEXT pair's iterations
                # so the DVE queue never blocks on the finalize chain
                ssums = {}

                def mk_ssum(g):
                    def run():
                        st0, st1 = hstages[g]
                        ssums[g] = fin2_pool.tile([65, QPC], f32, tag="ss",
                                                  name=f"ss{hp}{g}")
                        nc.vector.scalar_tensor_tensor(
                            ssums[g][:], st0[:], 1.0, st1[:],
                            ALU.mult, ALU.add)
                    return run

                def mk_chain(g, j):
                    def run():
                        h = 2 * hp + g
                        hpo = g * 64
                        pf = sc.tile([128, QPC], f32, tag="ps",
                                     name=f"pf{h}{j}")
                        nc.tensor.matmul(pf[:, 0:65],
                                         ssums[g][:, j * 128:(j + 1) * 128],
                                         identf[0:65, 0:65],
                                         is_transpose=True)
                        rec = fin2_pool.tile([128, 1], f32, tag="rec")
                        nc.vector.reciprocal(rec[:], pf[:, 64:65])
                        otmp = fin2_pool.tile([128, 64], f32, tag="ot")
                        if j % 2 == 0:
                            nc.scalar.mul(otmp[:], pf[:, 0:64], rec[:])
                        else:
                            nc.vector.tensor_scalar_mul(otmp[:], pf[:, 0:64],
                                                        rec[:])
                        ptr = sc.tile([128, QPC], f32, tag="ps",
                                      name=f"ptr{h}{j}")
                        nc.tensor.matmul(ptr[0:64, 0:128], otmp[:],
                                         identf[:], is_transpose=True)
                        nc.scalar.add(
                            qT[hpo:hpo + 64, hp, j * 128:(j + 1) * 128],
                            ptr[0:64, 0:128],
                            bias_vec[hpo:hpo + 64, hp, 2:3])
                    return run

                return ([mk_ssum(g) for g in range(2)]
                        + [mk_chain(g, j) for g in range(2)
                           for j in range(LTQ)])

            fi = 0

            def attention_pair(hp, bias_q, prev_fin, filler_budget):
                h0 = 2 * hp
                kTg = [kT[0:64, hp, :], kT[64:128, hp, :]]
                qTg = [qT[0:64, hp, :], qT[64:128, hp, :]]
                povs = {}
                hstages = {0: [], 1: []}
                pend = None
                nonlocal fi

                def emit_av(mt, egs):
                    half = mt // 8
                    for g in range(2):
                        if (g, half) not in povs:
                            povs[(g, half)] = po_pool.tile(
                                [65, QPC], f32, tag="po",
                                name=f"po{h0 + g}h{half}")
                        nc.tensor.matmul(
                            povs[(g, half)][:],
                            vsb[:, mt, (h0 + g) * 65:(h0 + g) * 65 + 65],
                            egs[g], start=(mt % 8 == 0),
                            stop=(mt % 8 == 7), skip_group_check=True)
                    if mt % 8 == 7:  # evict half-accumulators
                        for g in range(2):
                            st = fin_pool.tile([65, QPC], f32, tag="st",
                                               name=f"st{h0 + g}{half}")
                            eng = nc.vector if g == 0 else nc.scalar
                            if g == 0:
                                nc.vector.tensor_copy(st[:],
                                                      povs[(g, half)][:])
                            else:
                                nc.scalar.copy(st[:], povs[(g, half)][:])
                            hstages[g].append(st)

                for mt in range(MT):
                    if prev_fin is not None and mt == 2:
                        for piece in prev_fin:
                            piece()
                    if mt in (4, 7, 10, 13) and fi < len(fillers) \
                            and filler_budget > 0:
                        fillers[fi]()
                        fi += 1
                        filler_budget -= 1
                    if mt in (2, 5, 8, 11, 14):
                        # PE-injected bias: no DVE work for this key tile
                        e2 = e_pool.tile([128, 2, QPC], bf16, tag="e")
                        for g in range(2):
                            ps = sc.tile([128, QPC], f32, tag="ps")
                            bias_t = bias_q[(g, mt // 4)]
                            nc.tensor.matmul(ps[:], ident[:],
                                             bias_t[:, mt % 4, :],
                                             start=True, stop=False,
                                             skip_group_check=True)
                            nc.tensor.matmul(
                                ps[:], kTg[g][:, mt * 128:(mt + 1) * 128],
                                qTg[g][:], start=False, stop=True,
                                skip_group_check=True)
                            nc.scalar.activation(e2[:, g, :], ps[:], AF.Exp)
                        if pend is not None:
                            emit_av(*pend)
                        pend = (mt, [e2[:, 0, :], e2[:, 1, :]])
                        continue
                    sadd = sadd_pool.tile([128, 2, QPC], f32, tag="sa")
                    for g in range(2):
                        ps = sc.tile([128, QPC], f32, tag="ps")
                        nc.tensor.matmul(ps[:],
                                         kTg[g][:, mt * 128:(mt + 1) * 128],
                                         qTg[g][:], start=True, stop=True)
                        bias_t = bias_q[(g, mt // 4)]
                        nc.vector.scalar_tensor_tensor(
                            sadd[:, g, :], ps[:], 1.0,
                            bias_t[:, mt % 4, :], ALU.mult, ALU.add)
                    e2 = e_pool.tile([128, 2, QPC], bf16, tag="e")
                    nc.scalar.activation(e2[:], sadd[:], AF.Exp)
                    if pend is not None:
                        emit_av(*pend)
                    pend = (mt, [e2[:, 0, :], e2[:, 1, :]])
                emit_av(*pend)
                return finalize_pieces(hp, hstages)

            # bias quarter prefetch management: DMA pair hp's quarters just
            # before the pair runs; ring of 6 gives one-pair-ahead prefetch
            bias_store = {}

            def stage_bias(hp):
                m = {}
                for q in range(4):
                    for g in range(2):
                        m[(g, q)] = bias_dma(2 * hp + g, q)
                return m

            bias_store[0] = stage_bias(0)
            pending_fin = None
            for hp in range(KT):
                if hp + 1 < KT:
                    bias_store[hp + 1] = stage_bias(hp + 1)
                budget = 4 if hp < 4 else 2
                pending_fin = attention_pair(hp, bias_store.pop(hp),
                                             pending_fin, budget)
            while fi < len(fillers):
                fillers[fi]()
                fi += 1

            # ---- output projection y = O^T.T @ weffo + bo (OT aliased
            # into qT); the last pair's finalize chains are interleaved so
            # y's ki0-6 accumulation hides their latency ----
            pending_fin[0]()  # ssum g0
            pending_fin[1]()  # ssum g1
            for tt in range(LTQ):
                tsl = slice(tt * 128, (tt + 1) * 128)
                pys = [mm.tile([128, 512], f32, tag="mm",
                               name=f"py{tt}{oc}") for oc in range(2)]
                for ki in range(KT - 1):
                    for oc in range(2):
                        nc.tensor.matmul(
                            pys[oc], qT[:, ki, tsl],
                            weff["o"][:, ki, oc * 512:(oc + 1) * 512],
                            start=(ki == 0), stop=False,
                            skip_group_check=True)
                # finalize chains (g0, tt) and (g1, tt) produce OT ki=7
                # for exactly this token tile
                pending_fin[2 + tt]()
                pending_fin[2 + LTQ + tt]()
                ysb = y_pool.tile([128, D], bf16, tag="y")
                for oc in range(2):
                    osl = slice(oc * 512, (oc + 1) * 512)
                    nc.tensor.matmul(
                        pys[oc], qT[:, KT - 1, tsl],
                        weff["o"][:, KT - 1, osl],
                        start=False, stop=False, skip_group_check=True)
                    nc.tensor.matmul(pys[oc], ones_row[:], bo_row[:, osl],
                                     start=False, stop=True,
                                     skip_group_check=True)
                    nc.vector.tensor_copy(ysb[:, osl], pys[oc])
                nc.sync.dma_start(y_ap[tsl, :], ysb[:])

    nc.compile()
    return nc


def _shard_inputs(inputs):
    x = np.asarray(inputs["x"])
    bias = np.asarray(inputs["attn_bias"])
    # layout-only host prep: bf16 casts and transposes (no arithmetic)
    xT_bf = np.ascontiguousarray(
        x.astype(BF16).transpose(0, 2, 1))          # [B, D, L]
    biasT = np.ascontiguousarray(
        bias[0].astype(BF16).transpose(0, 2, 1))    # [H, m, l]
    shared = {}
    for p in "qkvo":
        shared[f"WT{p}"] = np.ascontiguousarray(
            inputs[f"W{p}"].astype(BF16).T)          # [in, out]
        shared[f"AT{p}"] = np.ascontiguousarray(
            inputs[f"A{p}"].astype(BF16).T)          # [R, D]
        shared[f"B{p}"] = inputs[f"B{p}"].astype(BF16)
    shared["bq"] = np.asarray(inputs["bq"], np.float32)[:, None]
    shared["bk"] = np.asarray(inputs["bk"], np.float32)[:, None]
    shared["bv2"] = np.asarray(inputs["bv"], np.float32)[:, None]
    shared["bo"] = np.asarray(inputs["bo"], np.float32)[None, :]
    in_maps = []
    for c in range(N_CORES):
        b, qb = divmod(c, 4)
        qsl = slice(qb * QPC, (qb + 1) * QPC)
        m = dict(shared)
        m["xbT"] = xT_bf[b]
        m["xqT"] = np.ascontiguousarray(xT_bf[b][:, qsl])
        m["biasT"] = np.ascontiguousarray(biasT[:, :, qsl])
        in_maps.append(m)
    return in_maps


def _gather_outputs(results):
    y = np.empty((B, L, D), np.float32)
    for c in range(N_CORES):
        b, qb = divmod(c, 4)
        y[b, qb * QPC:(qb + 1) * QPC] = results[c]["y"].astype(np.float32)
    return y


def get_nc(**kw):
    key = ("nc", tuple(sorted(kw.items())))
    if key not in _CACHE:
        _CACHE[key] = _build_kernel(**kw)
    return _CACHE[key]


def build_runner(nc, n_cores=N_CORES):
    """Jitted SPMD executable for a prebuilt Bass module."""
    import jax
    from jax.sharding import Mesh, PartitionSpec
    from jax.experimental.shard_map import shard_map
    import concourse.mybir as mybir
    from concourse.bass2jax import (_bass_exec_p, install_neuronx_cc_hook,
                                    partition_id_tensor)

    install_neuronx_cc_hook()
    partition_name = (nc.partition_id_tensor.name
                      if nc.partition_id_tensor else None)
    in_names, out_names, out_avals, zero_outs = [], [], [], []
    for alloc in nc.m.functions[0].allocations:
        if not isinstance(alloc, mybir.MemoryLocationSet):
            continue
        name = alloc.memorylocations[0].name
        if alloc.kind == "ExternalInput":
            if name != partition_name:
                in_names.append(name)
        elif alloc.kind == "ExternalOutput":
            shape = tuple(alloc.tensor_shape)
            dtype = mybir.dt.np(alloc.dtype)
            out_names.append(name)
            out_avals.append(jax.core.ShapedArray(shape, dtype))
            zero_outs.append(np.zeros(shape, dtype))
    n_params = len(in_names)
    n_outs = len(out_avals)
    all_in_names = list(in_names) + list(out_names)
    if partition_name is not None:
        all_in_names.append(partition_name)

    def _body(*args):
        operands = list(args)
        if partition_name is not None:
            operands.append(partition_id_tensor())
        outs = _bass_exec_p.bind(
            *operands,
            out_avals=tuple(out_avals),
            in_names=tuple(all_in_names),
            out_names=tuple(out_names),
            lowering_input_output_aliases=(),
            sim_require_finite=True,
            sim_require_nnan=True,
            nc=nc,
        )
        return tuple(outs)

    devices = jax.devices()[:n_cores]
    mesh = Mesh(np.asarray(devices), ("core",))
    in_specs = (PartitionSpec("core"),) * (n_params + n_outs)
    out_specs = (PartitionSpec("core"),) * n_outs
    fn = jax.jit(shard_map(_body, mesh=mesh, in_specs=in_specs,
                           out_specs=out_specs, check_rep=False),
                 keep_unused=True)
    return fn, in_names, out_names, zero_outs


def _get_runner():
    if "runner" not in _CACHE:
        _CACHE["runner"] = build_runner(get_nc())
    return _CACHE["runner"]


def run_on_device(in_maps):
    import jax
    fn, in_names, out_names, zero_outs = _get_runner()
    concat_in = [np.concatenate([np.asarray(in_maps[c][nm])
                                 for c in range(N_CORES)], axis=0)
                 for nm in in_names]
    concat_zeros = [np.zeros((N_CORES * z.shape[0], *z.shape[1:]), z.dtype)
                    for z in zero_outs]
    out = fn(*concat_in, *concat_zeros)
    jax.block_until_ready(out)
    results = []
    for c in range(N_CORES):
        d = {}
        for i, nm in enumerate(out_names):
            arr = np.asarray(out[i])
            per = arr.shape[0] // N_CORES
            d[nm] = arr[c * per:(c + 1) * per]
        results.append(d)
    return results


def kernel(**inputs) -> np.ndarray:
    in_maps = _shard_inputs(inputs)
    results = run_on_device(in_maps)
    return _gather_outputs(results)


# revision 17
# speedup vs baseline: 2.6089x; 2.6089x over previous
"""Multi-head self-attention with LoRA projections on 8 Trainium2 NeuronCores.

Problem: nn_MultiHeadSelfAttention (B=2, L=2048, D=1024, H=16, hd=64, LoRA r=16).

Sharding (ZERO-collective): query-token parallel. Core c owns the 512 query
tokens [qb*512,(qb+1)*512) of batch b, where b = c//4, qb = c%4. Each core
computes K/V for its whole batch locally (replicated across the 4 cores of
that batch group) — this trades ~55us of extra PE time for eliminating the
AllToAll collective (multiple ms in this environment) and makes the final
output a clean per-core concat. Weights/LoRA factors are replicated.

Per-core pipeline (bf16 on the PE, fp32 accumulation in PSUM):
  1. Weff_p = W_p^T + 0.5*A_p@B_p folded on-chip: W^T arrives as a host
     layout-transpose, the rank-16 LoRA product is a PE matmul accumulated
     in PSUM and added in-place on DVE. Attention scale 1/8 at Q eviction.
  2. kT = Weff_k^T x^T in [out, tok] layout (+bk at ACT eviction), streamed
     by 512-token x chunks; qT likewise for the core's own 512 tokens; V in
     [tok, out] layout with a ones column per head (softmax row sums); bv
     deferred past softmax (softmax rows sum to 1).
  3. Attention runs in HEAD PAIRS with two interleaved dependency chains:
     per key tile, S^T [m=128, l=512] in PSUM (contract hd=64); bias
     (host-pre-transposed [m,l] bf16) added on DVE for most tiles and
     PE-injected (identity-matmul accumulate) for 5/16 tiles to balance
     engines; one 1024-wide exp per pair on ACT; AV accumulates O'^T
     [65, 512] in half-chains (row 64 = softmax denominator) so the PSUM
     ring stays small. V's second half and the o-projection weight prep run
     as PE filler between pairs.
  4. Finalize per head: PE-transpose + DVE reciprocal normalize, transpose
     back, +bv; O^T overwrites the dead qT region (no extra SBUF).
  5. y = O^T.T @ Weff_o + bo (bo via rank-1 ones matmul), fp32 out.

Host side only shards/casts/layout-transposes/concats: slices x/bias per
core, casts to bf16, pre-transposes W/A/x/bias (layout only — all arithmetic
including the LoRA fold stays on device), concatenates the per-core
[512, 1024] fp32 outputs into [2, 2048, 1024].
"""

import numpy as np
import ml_dtypes

BF16 = ml_dtypes.bfloat16

B = 2
L = 2048
D = 1024
H = 16
HD = 64
R = 16
SCALING = 0.5  # LoRA alpha/r
SCALE = HD ** (-0.5)  # attention scale, applied at Q eviction

N_CORES = 8
QPC = 512  # query tokens per core
KT = D // 128  # 8 contraction tiles
MT = L // 128  # 16 key tiles per batch
LTQ = QPC // 128  # 4 query tiles per core

_CACHE = {}


def _build_kernel(num_devices=N_CORES, repeat=1):
    import concourse.tile as tile
    import concourse.mybir as mybir
    from concourse import bacc
    from concourse.masks import make_identity
    from contextlib import ExitStack

    f32 = mybir.dt.float32
    bf16 = mybir.dt.bfloat16
    AF = mybir.ActivationFunctionType
    ALU = mybir.AluOpType

    nc = bacc.Bacc("TRN2", target_bir_lowering=False, debug=False,
                   enable_asserts=False, num_devices=num_devices)

    # ---- per-core external inputs (bf16 pre-cast / layout-prepped on host) --
    xbT_ap = nc.dram_tensor("xbT", [D, L], bf16, kind="ExternalInput").ap()
    xqT_ap = nc.dram_tensor("xqT", [D, QPC], bf16, kind="ExternalInput").ap()
    biasT_ap = nc.dram_tensor("biasT", [H, L, QPC], bf16,
                              kind="ExternalInput").ap()
    wt_aps, at_aps, lb_aps = {}, {}, {}
    for p in "qkvo":
        wt_aps[p] = nc.dram_tensor(f"WT{p}", [D, D], bf16,
                                   kind="ExternalInput").ap()
        at_aps[p] = nc.dram_tensor(f"AT{p}", [R, D], bf16,
                                   kind="ExternalInput").ap()
        lb_aps[p] = nc.dram_tensor(f"B{p}", [R, D], bf16,
                                   kind="ExternalInput").ap()
    bq_ap = nc.dram_tensor("bq", [D, 1], f32, kind="ExternalInput").ap()
    bk_ap = nc.dram_tensor("bk", [D, 1], f32, kind="ExternalInput").ap()
    bv2_ap = nc.dram_tensor("bv2", [D, 1], f32, kind="ExternalInput").ap()
    bo_ap = nc.dram_tensor("bo", [1, D], f32, kind="ExternalInput").ap()

    y_ap = nc.dram_tensor("y", [QPC, D], bf16, kind="ExternalOutput").ap()

    with tile.TileContext(nc) as tc, ExitStack() as top:
        const_pool = top.enter_context(tc.tile_pool(name="const", bufs=1))
        ident = const_pool.tile([128, 128], bf16)
        make_identity(nc, ident[:])
        identf = const_pool.tile([128, 128], f32)
        make_identity(nc, identf[:])
        ones_row = const_pool.tile([1, 128], bf16)
        nc.gpsimd.memset(ones_row[:], 1.0)
        bias_vec = const_pool.tile([128, KT, 3], f32)
        # bias_vec[:, kt, 0..2] = bq*SCALE | bk | bv
        bo_row = const_pool.tile([1, D], bf16)

        for rep in range(repeat):
          with ExitStack() as rctx:
            qkv_pool = rctx.enter_context(tc.tile_pool(name="qkv", bufs=1))
            kT = qkv_pool.tile([128, KT, L], bf16)          # K^T [out, tok]
            vsb = qkv_pool.tile([128, MT, H * 65], bf16)    # V [tok, h|1]
            qT = qkv_pool.tile([128, KT, QPC], bf16)  # Q^T; becomes O^T+bv

            weff_pool = rctx.enter_context(tc.tile_pool(name="weff", bufs=3))
            lsm = rctx.enter_context(tc.tile_pool(name="lsm", bufs=1))
            lora_sm = rctx.enter_context(tc.tile_pool(name="lora", bufs=2))
            xts = rctx.enter_context(tc.tile_pool(name="xts", bufs=2))
            bias_pool = rctx.enter_context(tc.tile_pool(name="bias", bufs=5))
            sadd_pool = rctx.enter_context(tc.tile_pool(name="sadd", bufs=3))
            e_pool = rctx.enter_context(tc.tile_pool(name="e", bufs=4))
            fin_pool = rctx.enter_context(tc.tile_pool(name="fin", bufs=4))
            fin2_pool = rctx.enter_context(tc.tile_pool(name="fin2", bufs=2))
            y_pool = rctx.enter_context(tc.tile_pool(name="ysb", bufs=1))
            sc = rctx.enter_context(tc.tile_pool(name="scps", bufs=4,
                                                 space="PSUM"))
            mm = rctx.enter_context(tc.tile_pool(name="mmps", bufs=2,
                                                 space="PSUM"))
            po_pool = rctx.enter_context(tc.tile_pool(name="pops", bufs=2,
                                                      space="PSUM"))

            # vsb ones columns (disjoint from V evictions; runs immediately)
            ones_cols = vsb[:].rearrange("p m (h e) -> p m h e", e=65)
            nc.gpsimd.memset(ones_cols[:, :, :, 64:65], 1.0)

            ats, lb, weff = {}, {}, {}

            def lora_factors(p, eng=None):
                eng = eng or nc.sync
                lb[p] = lora_sm.tile([R, D], bf16, tag="lb", name=f"lb{p}{rep}")
                eng.dma_start(lb[p][:], lb_aps[p][:, :])
                araw = lora_sm.tile([R, D], bf16, tag="araw", name=f"ar{p}{rep}")
                eng.dma_start(araw[:], at_aps[p][:, :])
                ats[p] = lora_sm.tile([R, D], bf16, tag="ats", name=f"at{p}{rep}")
                nc.gpsimd.tensor_scalar_mul(ats[p][:], araw[:], SCALING)

            for p in "kqv":
                lora_factors(p)

            # ---- small DMAs ----
            braw = lsm.tile([128, KT, 3], f32, name=f"braw{rep}")
            nc.sync.dma_start(
                braw[:, :, 0:1], bq_ap.rearrange("(kt p) o -> p kt o", p=128))
            nc.sync.dma_start(
                braw[:, :, 1:2], bk_ap.rearrange("(kt p) o -> p kt o", p=128))
            nc.sync.dma_start(
                braw[:, :, 2:3], bv2_ap.rearrange("(kt p) o -> p kt o", p=128))
            nc.gpsimd.dma_start(bo_row[:], bo_ap[:, :])  # cast f32->bf16
            nc.vector.tensor_scalar_mul(bias_vec[:, :, 0:1],
                                        braw[:, :, 0:1], SCALE)
            nc.vector.tensor_copy(bias_vec[:, :, 1:3], braw[:, :, 1:3])

            def weff_dma(p):
                weff[p] = weff_pool.tile([128, KT, D], bf16, tag="we",
                                         name=f"we{p}{rep}")
                for ki in range(KT):
                    ksl = slice(ki * 128, (ki + 1) * 128)
                    nc.sync.dma_start(
                        weff[p][:, ki, :],
                        wt_aps[p][ksl, :].rearrange("(o p2) c -> p2 o c",
                                                    p2=128))

            def weff_fold(p):
                for ki in range(KT):
                    ksl = slice(ki * 128, (ki + 1) * 128)
                    for oc in range(2):
                        osl = slice(oc * 512, (oc + 1) * 512)
                        ps = mm.tile([128, 512], f32, tag="mm",
                                     name=f"wf{p}{ki}{oc}")
                        nc.tensor.matmul(ps[:], ats[p][:, ksl], lb[p][:, osl])
                        nc.vector.scalar_tensor_tensor(
                            weff[p][:, ki, osl], ps[:], 1.0,
                            weff[p][:, ki, osl], ALU.mult, ALU.add)

            # DMA queue order tuned so PE never waits long: Wk, x-chunk0,
            # xq, Wq, Wv, then remaining x chunks
            weff_dma("k")
            xc0 = xts.tile([128, KT, 512], bf16, tag="xc", name=f"xc0{rep}")
            nc.sync.dma_start(
                xc0[:], xbT_ap[:, 0:512].rearrange("(ki p2) t -> p2 ki t",
                                                   p2=128))
            xqT = xts.tile([128, KT, QPC], bf16, tag="xc", name=f"xq{rep}")
            nc.sync.dma_start(
                xqT[:], xqT_ap.rearrange("(ki p2) t -> p2 ki t", p2=128))
            weff_dma("q")
            weff_dma("v")

            weff_fold("k")

            def k_chunk(tcc, xTc):
                for ot in range(KT):
                    osl = slice(ot * 128, (ot + 1) * 128)
                    ps = mm.tile([128, 512], f32, tag="mm",
                                 name=f"pk{tcc}{ot}")
                    for ki in range(KT):
                        nc.tensor.matmul(ps[:], weff["k"][:, ki, osl],
                                         xTc[:, ki, :],
                                         start=(ki == 0), stop=(ki == KT - 1),
                                         skip_group_check=True)
                    nc.scalar.add(kT[:, ot, tcc * 512:(tcc + 1) * 512],
                                  ps[:], bias_vec[:, ot, 1:2])

            def v_chunk(tcc, xTc, oc):
                for tw in range(4):
                    tt = tcc * 4 + tw
                    twsl = slice(tw * 128, (tw + 1) * 128)
                    ps = mm.tile([128, 512], f32, tag="mm",
                                 name=f"pv{tt}{oc}")
                    for ki in range(KT):
                        nc.tensor.matmul(
                            ps[:], xTc[:, ki, twsl],
                            weff["v"][:, ki, oc * 512:(oc + 1) * 512],
                            start=(ki == 0), stop=(ki == KT - 1),
                            skip_group_check=True)
                    dst = vsb[:, tt, oc * 520:(oc + 1) * 520].rearrange(
                        "p (h e) -> p h e", e=65)
                    nc.scalar.copy(
                        dst[:, :, 0:64],
                        ps[:].rearrange("p (h e) -> p h e", e=64))

            k_chunk(0, xc0)
            weff_fold("q")
            for ot in range(KT):  # Q^T (scale + bq at ACT eviction)
                osl = slice(ot * 128, (ot + 1) * 128)
                ps = mm.tile([128, QPC], f32, tag="mm", name=f"pq{ot}")
                for ki in range(KT):
                    nc.tensor.matmul(ps[:], weff["q"][:, ki, osl],
                                     xqT[:, ki, :],
                                     start=(ki == 0), stop=(ki == KT - 1),
                                     skip_group_check=True)
                nc.scalar.activation(qT[:, ot, :], ps[:], AF.Identity,
                                     bias=bias_vec[:, ot, 0:1], scale=SCALE)
            weff_fold("v")
            v_chunk(0, xc0, 0)
            for tcc in range(1, 4):
                xTc = xts.tile([128, KT, 512], bf16, tag="xc",
                               name=f"xcA{tcc}")
                nc.sync.dma_start(
                    xTc[:],
                    xbT_ap[:, tcc * 512:(tcc + 1) * 512]
                    .rearrange("(ki p2) t -> p2 ki t", p2=128))
                k_chunk(tcc, xTc)
                v_chunk(tcc, xTc, 0)

            # ---- deferred filler jobs (run interleaved between heads) ----
            vstate = {}

            def v_oc1(tt):
                def run():
                    tcc, tw = divmod(tt, 4)
                    if tw == 0:
                        xc = xts.tile([128, KT, 512], bf16, tag="xc",
                                      name=f"xc1{tt}")
                        nc.gpsimd.dma_start(
                            xc[:],
                            xbT_ap[:, tcc * 512:(tcc + 1) * 512]
                            .rearrange("(ki p2) t -> p2 ki t", p2=128))
                        vstate["xc"] = xc
                    xc = vstate["xc"]
                    twsl = slice(tw * 128, (tw + 1) * 128)
                    ps = mm.tile([128, 512], f32, tag="mm", name=f"pw{tt}")
                    for ki in range(KT):
                        nc.tensor.matmul(ps[:], xc[:, ki, twsl],
                                         weff["v"][:, ki, 512:1024],
                                         start=(ki == 0), stop=(ki == KT - 1),
                                         skip_group_check=True)
                    dst = vsb[:, tt, 520:1040].rearrange("p (h e) -> p h e",
                                                         e=65)
                    nc.vector.tensor_copy(
                        dst[:, :, 0:64],
                        ps[:].rearrange("p (h e) -> p h e", e=64))
                return run

            def wo_load_fold_piece(ki):
                def run():
                    if ki == 0:
                        lora_factors("o", nc.gpsimd)
                        weff["o"] = weff_pool.tile([128, KT, D], bf16,
                                                   tag="we", name=f"weo{rep}")
                        # all chunk DMAs up front so later fold pieces
                        # never wait behind x-chunk transfers in the queue
                        for kj in range(KT):
                            kjs = slice(kj * 128, (kj + 1) * 128)
                            nc.gpsimd.dma_start(
                                weff["o"][:, kj, :],
                                wt_aps["o"][kjs, :].rearrange(
                                    "(o p2) c -> p2 o c", p2=128))
                    ksl = slice(ki * 128, (ki + 1) * 128)
                    for oc in range(2):
                        osl = slice(oc * 512, (oc + 1) * 512)
                        ps = mm.tile([128, 512], f32, tag="mm",
                                     name=f"wo{ki}{oc}")
                        nc.tensor.matmul(ps[:], ats["o"][:, ksl],
                                         lb["o"][:, osl])
                        nc.vector.scalar_tensor_tensor(
                            weff["o"][:, ki, osl], ps[:], 1.0,
                            weff["o"][:, ki, osl], ALU.mult, ALU.add)
                return run

            fillers = [v_oc1(tt) for tt in range(MT)]
            fillers += [wo_load_fold_piece(ki) for ki in range(KT)]

            # ---- attention: head PAIRS interleaved (two chains in flight),
            # AV split into half-accumulators so PSUM po ring stays at 2 ----
            def bias_dma(h, q):
                bt = bias_pool.tile([128, 4, QPC], bf16, tag="bn",
                                    name=f"bn{h}{q}")
                nc.sync.dma_start(
                    bt[:],
                    biasT_ap[h][q * 512:(q + 1) * 512, :]
                    .rearrange("(mt p) l -> p mt l", p=128))
                return bt

            def finalize_pieces(hp, hstages):
                # small closures (ssum per head, then per-j normalize
                # chains), emitted spread across the NEXT pair's iterations
                # so the DVE queue never blocks on the finalize chain
                ssums = {}

                def mk_ssum(g):
                    def run():
                        st0, st1 = hstages[g]
                        ssums[g] = fin2_pool.tile([65, QPC], f32, tag="ss",
                                                  name=f"ss{hp}{g}")
                        nc.vector.scalar_tensor_tensor(
                            ssums[g][:], st0[:], 1.0, st1[:],
                            ALU.mult, ALU.add)
                    return run

                def mk_chain(g, j):
                    def run():
                        h = 2 * hp + g
                        hpo = g * 64
                        pf = sc.tile([128, QPC], f32, tag="ps",
                                     name=f"pf{h}{j}")
                        nc.tensor.matmul(pf[:, 0:65],
                                         ssums[g][:, j * 128:(j + 1) * 128],
                                         identf[0:65, 0:65],
                                         is_transpose=True)
                        rec = fin2_pool.tile([128, 1], f32, tag="rec")
                        nc.vector.reciprocal(rec[:], pf[:, 64:65])
                        otmp = fin2_pool.tile([128, 64], f32, tag="ot")
                        if j % 2 == 0:
                            nc.scalar.mul(otmp[:], pf[:, 0:64], rec[:])
                        else:
                            nc.vector.tensor_scalar_mul(otmp[:], pf[:, 0:64],
                                                        rec[:])
                        ptr = sc.tile([128, QPC], f32, tag="ps",
                                      name=f"ptr{h}{j}")
                        nc.tensor.matmul(ptr[0:64, 0:128], otmp[:],
                                         identf[:], is_transpose=True)
                        nc.scalar.add(
                            qT[hpo:hpo + 64, hp, j * 128:(j + 1) * 128],
                            ptr[0:64, 0:128],
                            bias_vec[hpo:hpo + 64, hp, 2:3])
                    return run

                return ([mk_ssum(g) for g in range(2)]
                        + [mk_chain(g, j) for g in range(2)
                           for j in range(LTQ)])

            fi = 0

            def attention_pair(hp, bias_q, prev_fin, filler_budget):
                h0 = 2 * hp
                kTg = [kT[0:64, hp, :], kT[64:128, hp, :]]
                qTg = [qT[0:64, hp, :], qT[64:128, hp, :]]
                povs = {}
                hstages = {0: [], 1: []}
                pend = None
                nonlocal fi

                def emit_av(mt, egs):
                    half = mt // 8
                    for g in range(2):
                        if (g, half) not in povs:
                            povs[(g, half)] = po_pool.tile(
                                [65, QPC], f32, tag="po",
                                name=f"po{h0 + g}h{half}")
                        nc.tensor.matmul(
                            povs[(g, half)][:],
                            vsb[:, mt, (h0 + g) * 65:(h0 + g) * 65 + 65],
                            egs[g], start=(mt % 8 == 0),
                            stop=(mt % 8 == 7), skip_group_check=True)
                    if mt % 8 == 7:  # evict half-accumulators
                        for g in range(2):
                            st = fin_pool.tile([65, QPC], f32, tag="st",
                                               name=f"st{h0 + g}{half}")
                            eng = nc.vector if g == 0 else nc.scalar
                            if g == 0:
                                nc.vector.tensor_copy(st[:],
                                                      povs[(g, half)][:])
                            else:
                                nc.scalar.copy(st[:], povs[(g, half)][:])
                            hstages[g].append(st)

                for mt in range(MT):
                    if prev_fin is not None and mt == 2:
                        for piece in prev_fin:
                            piece()
                    if mt in (4, 7, 10, 13) and fi < len(fillers) \
                            and filler_budget > 0:
                        fillers[fi]()
                        fi += 1
                        filler_budget -= 1
                    if mt in (2, 5, 8, 11, 14):
                        # PE-injected bias: no DVE work for this key tile
                        e2 = e_pool.tile([128, 2, QPC], bf16, tag="e")
                        for g in range(2):
                            ps = sc.tile([128, QPC], f32, tag="ps")
                            bias_t = bias_q[(g, mt // 4)]
                            nc.tensor.matmul(ps[:], ident[:],
                                             bias_t[:, mt % 4, :],
                                             start=True, stop=False,
                                             skip_group_check=True)
                            nc.tensor.matmul(
                                ps[:], kTg[g][:, mt * 128:(mt + 1) * 128],
                                qTg[g][:], start=False, stop=True,
                                skip_group_check=True)
                            nc.scalar.activation(e2[:, g, :], ps[:], AF.Exp)
                        if pend is not None:
                            emit_av(*pend)
                        pend = (mt, [e2[:, 0, :], e2[:, 1, :]])
                        continue
                    sadd = sadd_pool.tile([128, 2, QPC], f32, tag="sa")
                    for g in range(2):
                        ps = sc.tile([128, QPC], f32, tag="ps")
                        nc.tensor.matmul(ps[:],
                                         kTg[g][:, mt * 128:(mt + 1) * 128],
                                         qTg[g][:], start=True, stop=True)
                        bias_t = bias_q[(g, mt // 4)]
                        nc.vector.scalar_tensor_tensor(
                            sadd[:, g, :], ps[:], 1.0,
                            bias_t[:, mt % 4, :], ALU.mult, ALU.add)
                    e2 = e_pool.tile([128, 2, QPC], bf16, tag="e")
                    nc.scalar.activation(e2[:], sadd[:], AF.Exp)
                    if pend is not None:
                        emit_av(*pend)
                    pend = (mt, [e2[:, 0, :], e2[:, 1, :]])
                emit_av(*pend)
                return finalize_pieces(hp, hstages)

            # bias quarter prefetch management: DMA pair hp's quarters just
            # before the pair runs; ring of 6 gives one-pair-ahead prefetch
            bias_store = {}

            def stage_bias(hp):
                m = {}
                for q in range(4):
                    for g in range(2):
                        m[(g, q)] = bias_dma(2 * hp + g, q)
                return m

            bias_store[0] = stage_bias(0)
            pending_fin = None
            for hp in range(KT):
                if hp + 1 < KT:
                    bias_store[hp + 1] = stage_bias(hp + 1)
                budget = 4 if hp < 4 else 2
                pending_fin = attention_pair(hp, bias_store.pop(hp),
                                             pending_fin, budget)
            while fi < len(fillers):
                fillers[fi]()
                fi += 1

            # ---- output projection y = O^T.T @ weffo + bo (OT aliased
            # into qT); the last pair's finalize chains are interleaved so
            # y's ki0-6 accumulation hides their latency ----
            pending_fin[0]()  # ssum g0
            pending_fin[1]()  # ssum g1
            for tt in range(LTQ):
                tsl = slice(tt * 128, (tt + 1) * 128)
                pys = [mm.tile([128, 512], f32, tag="mm",
                               name=f"py{tt}{oc}") for oc in range(2)]
                for ki in range(KT - 1):
                    for oc in range(2):
                        nc.tensor.matmul(
                            pys[oc], qT[:, ki, tsl],
                            weff["o"][:, ki, oc * 512:(oc + 1) * 512],
                            start=(ki == 0), stop=False,
                            skip_group_check=True)
                # finalize chains (g0, tt) and (g1, tt) produce OT ki=7
                # for exactly this token tile
                pending_fin[2 + tt]()
                pending_fin[2 + LTQ + tt]()
                ysb = y_pool.tile([128, D], bf16, tag="y")
                for oc in range(2):
                    osl = slice(oc * 512, (oc + 1) * 512)
                    nc.tensor.matmul(
                        pys[oc], qT[:, KT - 1, tsl],
                        weff["o"][:, KT - 1, osl],
                        start=False, stop=False, skip_group_check=True)
                    nc.tensor.matmul(pys[oc], ones_row[:], bo_row[:, osl],
                                     start=False, stop=True,
                                     skip_group_check=True)
                    nc.vector.tensor_copy(ysb[:, osl], pys[oc])
                nc.sync.dma_start(y_ap[tsl, :], ysb[:])

    nc.compile()
    return nc


def _shard_inputs(inputs):
    x = np.asarray(inputs["x"])
    bias = np.asarray(inputs["attn_bias"])
    # layout-only host prep: bf16 casts and transposes (no arithmetic)
    xT_bf = np.ascontiguousarray(
        x.astype(BF16).transpose(0, 2, 1))          # [B, D, L]
    biasT = np.ascontiguousarray(
        bias[0].astype(BF16).transpose(0, 2, 1))    # [H, m, l]
    shared = {}
    for p in "qkvo":
        shared[f"WT{p}"] = np.ascontiguousarray(
            inputs[f"W{p}"].astype(BF16).T)          # [in, out]
        shared[f"AT{p}"] = np.ascontiguousarray(
            inputs[f"A{p}"].astype(BF16).T)          # [R, D]
        shared[f"B{p}"] = inputs[f"B{p}"].astype(BF16)
    shared["bq"] = np.asarray(inputs["bq"], np.float32)[:, None]
    shared["bk"] = np.asarray(inputs["bk"], np.float32)[:, None]
    shared["bv2"] = np.asarray(inputs["bv"], np.float32)[:, None]
    shared["bo"] = np.asarray(inputs["bo"], np.float32)[None, :]
    in_maps = []
    for c in range(N_CORES):
        b, qb = divmod(c, 4)
        qsl = slice(qb * QPC, (qb + 1) * QPC)
        m = dict(shared)
        m["xbT"] = xT_bf[b]
        m["xqT"] = np.ascontiguousarray(xT_bf[b][:, qsl])
        m["biasT"] = np.ascontiguousarray(biasT[:, :, qsl])
        in_maps.append(m)
    return in_maps


def _gather_outputs(results):
    y = np.empty((B, L, D), np.float32)
    for c in range(N_CORES):
        b, qb = divmod(c, 4)
        y[b, qb * QPC:(qb + 1) * QPC] = results[c]["y"].astype(np.float32)
    return y


def get_nc(**kw):
    key = ("nc", tuple(sorted(kw.items())))
    if key not in _CACHE:
        _CACHE[key] = _build_kernel(**kw)
    return _CACHE[key]


def build_runner(nc, n_cores=N_CORES):
    """Jitted SPMD executable for a prebuilt Bass module."""
    import jax
    from jax.sharding import Mesh, PartitionSpec
    from jax.experimental.shard_map import shard_map
    import concourse.mybir as mybir
    from concourse.bass2jax import (_bass_exec_p, install_neuronx_cc_hook,
                                    partition_id_tensor)

    install_neuronx_cc_hook()
    partition_name = (nc.partition_id_tensor.name
                      if nc.partition_id_tensor else None)
    in_names, out_names, out_avals, zero_outs = [], [], [], []
    for alloc in nc.m.functions[0].allocations:
        if not isinstance(alloc, mybir.MemoryLocationSet):
            continue
        name = alloc.memorylocations[0].name
        if alloc.kind == "ExternalInput":
            if name != partition_name:
                in_names.append(name)
        elif alloc.kind == "ExternalOutput":
            shape = tuple(alloc.tensor_shape)
            dtype = mybir.dt.np(alloc.dtype)
            out_names.append(name)
            out_avals.append(jax.core.ShapedArray(shape, dtype))
            zero_outs.append(np.zeros(shape, dtype))
    n_params = len(in_names)
    n_outs = len(out_avals)
    all_in_names = list(in_names) + list(out_names)
    if partition_name is not None:
        all_in_names.append(partition_name)

    def _body(*args):
        operands = list(args)
        if partition_name is not None:
            operands.append(partition_id_tensor())
        outs = _bass_exec_p.bind(
            *operands,
            out_avals=tuple(out_avals),
            in_names=tuple(all_in_names),
            out_names=tuple(out_names),
            lowering_input_output_aliases=(),
            sim_require_finite=True,
            sim_require_nnan=True,
            nc=nc,
        )
        return tuple(outs)

    devices = jax.devices()[:n_cores]
    mesh = Mesh(np.asarray(devices), ("core",))
    in_specs = (PartitionSpec("core"),) * (n_params + n_outs)
    out_specs = (PartitionSpec("core"),) * n_outs
    fn = jax.jit(shard_map(_body, mesh=mesh, in_specs=in_specs,
                           out_specs=out_specs, check_rep=False),
                 keep_unused=True)
    return fn, in_names, out_names, zero_outs


def _get_runner():
    if "runner" not in _CACHE:
        _CACHE["runner"] = build_runner(get_nc())
    return _CACHE["runner"]


def run_on_device(in_maps):
    import jax
    fn, in_names, out_names, zero_outs = _get_runner()
    concat_in = [np.concatenate([np.asarray(in_maps[c][nm])
                                 for c in range(N_CORES)], axis=0)
                 for nm in in_names]
    concat_zeros = [np.zeros((N_CORES * z.shape[0], *z.shape[1:]), z.dtype)
                    for z in zero_outs]
    out = fn(*concat_in, *concat_zeros)
    jax.block_until_ready(out)
    results = []
    for c in range(N_CORES):
        d = {}
        for i, nm in enumerate(out_names):
            arr = np.asarray(out[i])
            per = arr.shape[0] // N_CORES
            d[nm] = arr[c * per:(c + 1) * per]
        results.append(d)
    return results


def kernel(**inputs) -> np.ndarray:
    in_maps = _shard_inputs(inputs)
    results = run_on_device(in_maps)
    return _gather_outputs(results)
